# revision 1
# baseline (speedup 1.0000x reference)
"""Multi-head self-attention (16 heads, fake-quantized projections) on 8 trn2 cores.

Sharding: core c handles batch b = c // 4 and head group hg = c % 4 (global
heads 4*hg .. 4*hg+3). Each core computes its 4 heads' attention and a partial
output projection [S, E]; the host sums the 4 partials per batch.

Device pipeline per core (all big matmuls in float32r = 20-bit fp, 1 cyc/row):
  1. x^T tiles [e, s] DMA'd from a host-pretransposed f32r plane.
  2. q^T/k^T = W^T @ x^T in an interleaved layout [4h x d_lo | 4h x d_hi] so
     RoPE runs as full-width DVE ops; v in natural [s, d] layout with x^T as
     the stationary operand. RoPE outputs f32r.
  3. SBUF->SBUF DMA rearrange of rotated q/k into head-contiguous [2h x 64d]
     tiles so score matmuls contract K=64 in a single matmul.
  4. scores^T [kpos, q] = k^T.T @ q^T per head; exp on ScalarE with fused
     scale=1/8 (softmax max-subtraction skipped: scores ~ N(0,1), |s|/8 < 10).
  5. PV: U^T[d, q] plus a denominator row via a ones-column appended to V,
     accumulated over kpos tiles in PSUM.
  6. normalize: denominator row -> partition 0 via DMA, reciprocal (DVE),
     broadcast to 64 partitions via a K=1 fp32 matmul, fused multiply on the
     PSUM->SBUF copy.
  7. y_partial = sum_h U_norm^T.T @ Wout_h (K=64 accumulating matmuls).
Weights fake-quantized on host (exact numpy replica of the reference) and
pre-rounded to f32r (RNE).
"""
import sys, types
import numpy as np

sys.path.insert(0, "/opt/trn_rl_repo")

# NTFF profile hook shim (stub antenv package lacks axon_hooks; harmless if absent)
try:
    from trn_agent_boot.trn_boot import _ntff_profile_via_ctypes
    _hook = _ntff_profile_via_ctypes("/opt/axon/libaxon_pjrt.so")
    _m = types.ModuleType("antenv.axon_hooks")
    _m.get_axon_ntff_profile_hook = lambda: _hook
    _m.set_axon_ntff_profile_hook = lambda h: None
    sys.modules.setdefault("antenv.axon_hooks", _m)
except Exception:
    pass

import ml_dtypes
import concourse.bacc as bacc
import concourse.tile as tile
from concourse import mybir
from concourse import bass_utils as _bu
_bu.upload_artifacts = lambda tmpdir: "local://" + tmpdir

F32 = mybir.dt.float32
F32R = mybir.dt.float32r
BF16 = mybir.dt.bfloat16
AF = mybir.ActivationFunctionType

B, S, E = 2, 2048, 1024
H, D = 16, 64
HL = 4          # heads per core
ET = E // 128   # 8 e-tiles
ST = S // 128   # 16 s-tiles
QC = 1024       # attention q-chunk
NQC = S // QC
KT = S // 128   # 16 kpos tiles
SH = 1024       # s processed in halves in the projection phase


def round_f32r(x):
    """RNE round of fp32 to 1s + 8e + 11m (fp32r)."""
    xi = np.ascontiguousarray(x, dtype=np.float32).view(np.uint32).astype(np.uint64)
    lsb = (xi >> 12) & 1
    xi = xi + ((1 << 11) - 1 + lsb)
    xi = (xi >> 12) << 12
    return xi.astype(np.uint32).view(np.float32)


def quantize_bits_np(x):
    """Exact numpy replica of reference.quantize_bits(x, 8) in float32."""
    x = np.asarray(x, dtype=np.float32)
    qmax = np.float32(255.0)
    x_min = x.min()
    x_max = x.max()
    scale = np.float32((x_max - x_min) / np.float32(qmax + np.float32(1e-8)))
    x_q = np.round(np.clip((x - x_min) / np.float32(scale + np.float32(1e-8)),
                           np.float32(0.0), qmax)).astype(np.float32)
    return x_q * scale + x_min


def rope_tables():
    inv_freq = (1.0 / 10000.0 ** (np.arange(0, D, 2, dtype=np.float32) / D)).astype(np.float32)
    t = np.arange(S, dtype=np.float32)
    freqs = t[:, None].astype(np.float32) * inv_freq[None, :]
    sin = np.sin(freqs).astype(np.float32)   # (S, 32)
    cos = np.cos(freqs).astype(np.float32)
    cosT = np.tile(np.ascontiguousarray(cos.T), (4, 1))  # (128, S), [d, s]
    sinT = np.tile(np.ascontiguousarray(sin.T), (4, 1))
    return cosT, sinT


def build_kernel(debug=False):
    nc = bacc.Bacc(trn_type="TRN2")
    dbg = {}
    if debug:
        for name, shape in [("d_xt0", [128, SH]), ("d_qc0", [128, S]),
                            ("d_kc0", [128, S]), ("d_va0", [128, HL * (D + 1)]),
                            ("d_p000", [128, QC]), ("d_bc00", [D, QC]),
                            ("d_un0", [D, S]), ("d_inv0", [1, QC])]:
            dbg[name] = nc.declare_dram_parameter(name, shape, F32, isOutput=True)
    xt = nc.declare_dram_parameter("xt", [E, S], F32R, isOutput=False)
    wqk = nc.declare_dram_parameter("wqk", [4, E, 128], F32R, isOutput=False)
    wv = nc.declare_dram_parameter("wv", [E, HL * D], F32R, isOutput=False)
    wout = nc.declare_dram_parameter("wout", [HL, D, E], F32R, isOutput=False)
    cost = nc.declare_dram_parameter("cost", [128, S], F32, isOutput=False)
    sint = nc.declare_dram_parameter("sint", [128, S], F32, isOutput=False)
    ypart = nc.declare_dram_parameter("ypart", [S, E], F32, isOutput=True)

    with tile.TileContext(nc) as tc:
        with (
            tc.tile_pool(name="sb", bufs=1) as sb,
            tc.tile_pool(name="ps", bufs=2, space="PSUM") as ps,
        ):
            # ---------------- constants
            # ones row lives at partition 64 to match the denominator row of
            # the PV accumulator (engine lanes are partition-locked)
            ones65 = sb.tile([D + 1, D], F32R, tag="ones", bufs=1)
            nc.vector.memset(ones65.bitcast(F32), 1.0)
            # cos/sin share the big "roam" slots ([128, 2048] fp32-sized)
            cos_sb = sb.tile([128, S], F32, tag="roam", bufs=8)
            sin_sb = sb.tile([128, S], F32, tag="roam", bufs=8)
            nc.sync.dma_start(out=cos_sb, in_=cost[:, :])
            nc.sync.dma_start(out=sin_sb, in_=sint[:, :])
            wqk_sb = {}
            for ct in range(4):
                for et in range(ET):
                    t = sb.tile([128, 128], F32R, tag="wqk", bufs=32,
                                name=f"wqk{ct}_{et}")
                    nc.sync.dma_start(out=t, in_=wqk[ct, et * 128:(et + 1) * 128, :])
                    wqk_sb[(ct, et)] = t
            wv_sb = []
            for et in range(ET):
                t = sb.tile([128, HL * D], F32R, tag="wv", bufs=8, name=f"wv{et}")
                nc.sync.dma_start(out=t, in_=wv[et * 128:(et + 1) * 128, :])
                wv_sb.append(t)

            # head-contiguous rotated q/k and per-head attention outputs
            qcont = [sb.tile([128, S], F32R, tag="roam", bufs=8, name=f"qcont{p}")
                     for p in range(2)]
            kcont = [sb.tile([128, S], F32R, tag="roam", bufs=8, name=f"kcont{p}")
                     for p in range(2)]
            u_norm = [sb.tile([D, S], F32R, tag="roam", bufs=8, name=f"unorm{h}")
                      for h in range(HL)]
            v_aug = []

            # ---------------- phases 1-2 per s-half: x^T, q/k proj + RoPE, v
            for sh in range(S // SH):
                ssl = slice(sh * SH, (sh + 1) * SH)
                xT = []
                for et in range(ET):
                    t = sb.tile([128, SH], F32R, tag="xT", bufs=ET, name=f"xT{et}")
                    nc.sync.dma_start(out=t, in_=xt[et * 128:(et + 1) * 128, ssl])
                    if debug and sh == 0 and et == 0:
                        nc.sync.dma_start(out=dbg["d_xt0"][:, :], in_=t.bitcast(F32))
                    xT.append(t)

                # q/k projection in interleaved layout + RoPE + rearrange
                for pair, dest in ((0, "q"), (2, "k")):
                    rot = [sb.tile([128, SH], F32R, tag="rot", bufs=2,
                                   name=f"rot{dest}{half}{sh}") for half in range(2)]
                    for q2 in range(SH // 512):
                        qsl = slice(q2 * 512, (q2 + 1) * 512)
                        gsl = slice(sh * SH + q2 * 512, sh * SH + (q2 + 1) * 512)
                        bb = ps.tile([128, 1024], F32, tag="sc", bufs=2)
                        b1, b2 = bb[:, 0:512], bb[:, 512:1024]
                        for et in range(ET):
                            nc.tensor.matmul(b1, wqk_sb[(pair, et)], xT[et][:, qsl],
                                             start=(et == 0), stop=(et == ET - 1))
                        for et in range(ET):
                            nc.tensor.matmul(b2, wqk_sb[(pair + 1, et)], xT[et][:, qsl],
                                             start=(et == 0), stop=(et == ET - 1))
                        t1 = sb.tile([128, 512], F32, tag="t1", bufs=2)
                        t2 = sb.tile([128, 512], F32, tag="t2", bufs=2)
                        nc.vector.tensor_mul(t1, b1, cos_sb[:, gsl])
                        nc.vector.tensor_mul(t2, b2, sin_sb[:, gsl])
                        nc.vector.tensor_sub(rot[0][:, qsl], t1, t2)
                        t3 = sb.tile([128, 512], F32, tag="t1", bufs=2)
                        t4 = sb.tile([128, 512], F32, tag="t2", bufs=2)
                        nc.vector.tensor_mul(t3, b1, sin_sb[:, gsl])
                        nc.vector.tensor_mul(t4, b2, cos_sb[:, gsl])
                        nc.vector.tensor_add(rot[1][:, qsl], t3, t4)
                    cont = qcont if dest == "q" else kcont
                    for h in range(HL):
                        p, j = divmod(h, 2)
                        for half in range(2):
                            rows_out = slice(64 * j + 32 * half, 64 * j + 32 * half + 32)
                            nc.sync.dma_start(
                                out=cont[p][rows_out, ssl],
                                in_=rot[half][32 * h:32 * h + 32, :])

                # v projection (natural [s, d]) + ones column
                for st_l in range(SH // 128):
                    st = sh * (SH // 128) + st_l
                    pv = ps.tile([128, HL * D], F32, tag="sc", bufs=2)
                    for et in range(ET):
                        nc.tensor.matmul(pv, xT[et][:, st_l * 128:(st_l + 1) * 128],
                                         wv_sb[et], start=(et == 0), stop=(et == ET - 1))
                    va = sb.tile([128, HL, D + 1], F32R, tag="vaug", bufs=ST,
                                 name=f"vaug{st}")
                    nc.vector.memset(va.bitcast(F32), 1.0)
                    nc.vector.tensor_copy(va[:, :, 0:D],
                                          pv.rearrange("p (h d) -> p h d", h=HL))
                    if debug and st == 0:
                        nc.sync.dma_start(out=dbg["d_va0"][:, :],
                                          in_=va.bitcast(F32).rearrange("p a b -> p (a b)"))
                    v_aug.append(va)

            # ---------------- phase 3: attention
            # qchunk 512, PVs skewed one kt behind the scores so exps hide
            # under the next kt's scores and the in-order PE queue never
            # stalls (stalls re-throttle the PE clock to 1.2 GHz).
            # U accumulators are evicted to SBUF right after their last PV so
            # 8 PSUM banks suffice; normalization (reciprocal in place,
            # K=1 broadcast matmul, fused multiply) happens far downstream.
            from concourse.tile_rust import add_dep_helper

            def finish_norm(h, qc, u_raw):
                qsl = slice(qc * 512, (qc + 1) * 512)
                bc = ps.tile([D, 512], F32, tag="sc", bufs=2)
                nc.tensor.matmul(bc, ones65[D:D + 1, :], u_raw[D:D + 1, :],
                                 start=True, stop=True)
                bc_sb = sb.tile([D, 512], F32, tag="bcsb", bufs=2)
                nc.vector.tensor_copy(bc_sb, bc)
                if debug and qc == 0 and h == 0:
                    nc.sync.dma_start(out=dbg["d_bc00"][:, 0:512], in_=bc_sb)
                    nc.sync.dma_start(out=dbg["d_inv0"][:, 0:512],
                                      in_=u_raw[D:D + 1, :].bitcast(F32))
                nc.vector.tensor_mul(u_norm[h][:, qsl], u_raw[0:D, :], bc_sb)

            NQ = S // 512
            skewed = None      # (qc, upsum, p_ts of previous kt)
            to_evict = None    # (qc, upsum) finished accumulating
            to_finish = []     # (h, qc, u_raw) awaiting normalization
            last_pv = None
            for qc in range(NQ):
                qsl = slice(qc * 512, (qc + 1) * 512)
                upsum = {}
                for kt in range(KT):
                    # scores + exp for (qc, kt)
                    p_ts = {}
                    s_first, s_last = None, None
                    for pr in range(2):
                        s_ps = ps.tile([128, 1024], F32, tag="sc", bufs=2)
                        for j in range(2):
                            mm = nc.tensor.matmul(
                                s_ps[:, j * 512:(j + 1) * 512],
                                kcont[pr][64 * j:64 * j + 64, kt * 128:(kt + 1) * 128],
                                qcont[pr][64 * j:64 * j + 64, qsl],
                                start=True, stop=True)
                            if last_pv is not None:
                                add_dep_helper(mm.ins, last_pv.ins, sync=False,
                                               reason="pe order")
                            s_last = mm
                        p_t = sb.tile([128, 1024], F32R, tag="p", bufs=4)
                        nc.scalar.activation(p_t, s_ps, AF.Exp, scale=0.125)
                        if debug and qc == 0 and pr == 0 and kt == 0:
                            nc.sync.dma_start(out=dbg["d_p000"][:, :],
                                              in_=p_t.bitcast(F32))
                        p_ts[pr] = p_t
                    # evict the previous q-chunk's finished U accumulators
                    if to_evict is not None:
                        eqc, eup = to_evict
                        for h in range(HL):
                            u_raw = sb.tile([D + 1, 512], F32R, tag="uraw", bufs=8,
                                            name=f"uraw{h}_{eqc}")
                            nc.vector.tensor_copy(u_raw, eup[h])
                            with nc.allow_low_precision(reason="denom recip"):
                                nc.vector.reciprocal(u_raw[D:D + 1, :],
                                                     u_raw[D:D + 1, :])
                            to_finish.append((h, eqc, u_raw))
                        to_evict = None
                    # skewed PV batch for (qc, kt-1) / (qc-1, KT-1)
                    if skewed is not None:
                        sqc, sup, sp = skewed
                        if not sup:
                            for h in range(HL):
                                sup[h] = ps.tile([D + 1, 512], F32, tag="pv",
                                                 bufs=4, name=f"u{h}_{sqc}")
                        for h in range(HL):
                            mm = nc.tensor.matmul(
                                sup[h], v_aug[sp[1]][:, h, :],
                                sp[0][h // 2][:, (h % 2) * 512:(h % 2) * 512 + 512],
                                start=(sp[1] == 0), stop=(sp[1] == KT - 1))
                            add_dep_helper(mm.ins, s_last.ins, sync=False,
                                           reason="pe order")
                            last_pv = mm
                        if sp[1] == KT - 1:
                            to_evict = (sqc, sup)
                    # deferred normalizations, far from their reciprocals
                    if kt == 8 and to_finish:
                        for args in to_finish[:4]:
                            finish_norm(*args)
                        to_finish = to_finish[4:]
                    skewed = (qc, upsum, (p_ts, kt))
            # drain: last PV batch, evict, normalize
            sqc, sup, sp = skewed
            for h in range(HL):
                mm = nc.tensor.matmul(
                    sup[h], v_aug[sp[1]][:, h, :],
                    sp[0][h // 2][:, (h % 2) * 512:(h % 2) * 512 + 512],
                    start=False, stop=True)
            for h in range(HL):
                u_raw = sb.tile([D + 1, 512], F32R, tag="uraw", bufs=8,
                                name=f"uraw{h}_{sqc}")
                nc.vector.tensor_copy(u_raw, sup[h])
                with nc.allow_low_precision(reason="denom recip"):
                    nc.vector.reciprocal(u_raw[D:D + 1, :], u_raw[D:D + 1, :])
                to_finish.append((h, sqc, u_raw))
            for args in to_finish:
                finish_norm(*args)

            if debug:
                nc.sync.dma_start(out=dbg["d_qc0"][:, :], in_=qcont[0].bitcast(F32))
                nc.sync.dma_start(out=dbg["d_kc0"][:, :], in_=kcont[0].bitcast(F32))
                nc.sync.dma_start(out=dbg["d_un0"][:, :], in_=u_norm[0].bitcast(F32))

            # ---------------- phase 4: output projection (partial)
            for ec in range(2):
                esl = slice(ec * 512, (ec + 1) * 512)
                wo_e = []
                for h in range(HL):
                    t = sb.tile([D, 512], F32R, tag="woute", bufs=8, name=f"wo{h}_{ec}")
                    nc.sync.dma_start(out=t, in_=wout[h, :, esl])
                    wo_e.append(t)
                for st in range(ST):
                    y_ps = ps.tile([128, 512], F32, tag="sc", bufs=2)
                    for h in range(HL):
                        nc.tensor.matmul(y_ps, u_norm[h][:, st * 128:(st + 1) * 128],
                                         wo_e[h], start=(h == 0), stop=(h == HL - 1))
                    y_sb = sb.tile([128, 512], F32, tag="ysb", bufs=1)
                    nc.vector.tensor_copy(y_sb, y_ps)
                    nc.sync.dma_start(out=ypart[st * 128:(st + 1) * 128, esl],
                                      in_=y_sb)
    nc.finalize()
    return nc


def make_inputs(x, w_qkv, w_out):
    """Host-side prep: quantize, round to f32r, split/re-layout per core."""
    x = np.asarray(x, dtype=np.float32)
    wq_deq = round_f32r(quantize_bits_np(np.asarray(w_qkv, dtype=np.float32)))
    wo_deq = round_f32r(quantize_bits_np(np.asarray(w_out, dtype=np.float32)))
    cosT, sinT = rope_tables()

    x_t = [round_f32r(np.ascontiguousarray(x[b].T)) for b in range(B)]

    in_maps = []
    for c in range(8):
        b, hg = divmod(c, 4)
        heads = [hg * HL + i for i in range(HL)]
        # interleaved q/k col-tiles [4, E, 128]: 0=q d_lo, 1=q d_hi, 2=k d_lo, 3=k d_hi
        wqk_t = np.empty((4, E, 128), dtype=np.float32)
        for half in range(2):
            cols = np.concatenate(
                [np.arange(h * D + 32 * half, h * D + 32 * half + 32) for h in heads])
            wqk_t[0 + half] = wq_deq[:, 0 * E + cols]
            wqk_t[2 + half] = wq_deq[:, 1 * E + cols]
        vcols = np.concatenate([np.arange(h * D, h * D + D) for h in heads])
        wv_t = np.ascontiguousarray(wq_deq[:, 2 * E + vcols])
        wout_t = np.stack([wo_deq[h * D:(h + 1) * D, :] for h in heads])
        in_maps.append({
            "xt": x_t[b],
            "wqk": wqk_t, "wv": wv_t, "wout": wout_t,
            "cost": cosT, "sint": sinT,
        })
    return in_maps


_NC_CACHE = {}


def get_nc():
    if "nc" not in _NC_CACHE:
        _NC_CACHE["nc"] = build_kernel()
    return _NC_CACHE["nc"]


def kernel(x, w_qkv, w_out):
    from concourse.bass_utils import run_bass_kernel_spmd
    nc = get_nc()
    in_maps = make_inputs(x, w_qkv, w_out)
    res = run_bass_kernel_spmd(nc, in_maps, list(range(8)))
    out = np.zeros((B, S, E), dtype=np.float32)
    for c in range(8):
        out[c // 4] += res.results[c]["ypart"]
    return out



# revision 9
# speedup vs baseline: 1.6838x; 1.6838x over previous
"""Multi-head self-attention (16 heads, fake-quantized projections) on 8 trn2 cores.

Sharding: core c handles batch b = c // 4 and head group hg = c % 4 (global
heads 4*hg .. 4*hg+3). Each core computes its 4 heads' attention and a partial
output projection [S, E]; the host sums the 4 partials per batch.

v2 pipeline (vs baseline): bf16 attention path (q/k/v/p), packed weight DMAs,
K=128 paired output projection, fast approximate reciprocal, PE-dense
scheduling. ScalarE exp (~16.8M elements) is the roofline engine.

Device pipeline per core:
  1. x^T tiles [e, s] from a host-pretransposed bf16 plane (full prefetch).
  2. q^T/k^T = W^T @ x^T in interleaved layout [4h x d_lo | 4h x d_hi] so
     RoPE runs as full-width DVE ops; v in natural [s, d] layout. RoPE
     outputs bf16.
  3. SBUF->SBUF DMA rearrange of rotated q/k into head-contiguous
     [2h x 64d] tiles so score matmuls contract K=64 in one matmul
     (row-pairs run concurrently on the PE array via base-partition tiling).
  4. scores^T [kpos, q] = k^T.T @ q^T per head; exp on ScalarE with fused
     scale=1/8 (softmax max-subtraction skipped: scores ~ N(0,1), |s|/8 < 10).
  5. PV: U^T[d, q] plus a denominator row via a ones-column appended to V,
     accumulated over kpos tiles in PSUM; PVs skewed one kt behind scores.
  6. normalize: fast reciprocal (custom DVE op) on the denominator row in
     place, K=1 broadcast matmul, multiply into stacked pair tiles
     u2[hp] [128=2x64d, S] (odd heads hop partitions via SBUF->SBUF DMA).
  7. y_partial = sum_hp u2[hp].T @ Wout_pair (K=128 accumulating matmuls);
     evictions alternate VectorE/ScalarE; DMA out per 128-row stripe.
Weights fake-quantized on host (exact numpy replica of the reference);
attention-path operands rounded to bf16, output projection kept in f32r.
"""
import sys, types
import numpy as np

sys.path.insert(0, "/opt/trn_rl_repo")

# NTFF profile hook shim (stub antenv package lacks axon_hooks; harmless if absent)
try:
    from trn_agent_boot.trn_boot import _ntff_profile_via_ctypes
    _hook = _ntff_profile_via_ctypes("/opt/axon/libaxon_pjrt.so")
    _m = types.ModuleType("antenv.axon_hooks")
    _m.get_axon_ntff_profile_hook = lambda: _hook
    _m.set_axon_ntff_profile_hook = lambda h: None
    sys.modules.setdefault("antenv.axon_hooks", _m)
except Exception:
    pass

import ml_dtypes
import concourse.bacc as bacc
import concourse.tile as tile
from concourse import mybir
from concourse import bass_utils as _bu
_bu.upload_artifacts = lambda tmpdir: "local://" + tmpdir

F32 = mybir.dt.float32
F32R = mybir.dt.float32r
BF16 = mybir.dt.bfloat16
AF = mybir.ActivationFunctionType

B, S, E = 2, 2048, 1024
H, D = 16, 64
HL = 4          # heads per core
ET = E // 128   # 8 e-tiles
ST = S // 128   # 16 s-tiles
KT = S // 128   # 16 kpos tiles
SH = 1024       # s processed in halves in the projection phase
NQ = 4          # 512-wide attention q-chunks


def round_f32r(x):
    """RNE round of fp32 to 1s + 8e + 11m (fp32r)."""
    xi = np.ascontiguousarray(x, dtype=np.float32).view(np.uint32).astype(np.uint64)
    lsb = (xi >> 12) & 1
    xi = xi + ((1 << 11) - 1 + lsb)
    xi = (xi >> 12) << 12
    return xi.astype(np.uint32).view(np.float32)


def quantize_bits_np(x):
    """Exact numpy replica of reference.quantize_bits(x, 8) in float32."""
    x = np.asarray(x, dtype=np.float32)
    qmax = np.float32(255.0)
    x_min = x.min()
    x_max = x.max()
    scale = np.float32((x_max - x_min) / np.float32(qmax + np.float32(1e-8)))
    x_q = np.round(np.clip((x - x_min) / np.float32(scale + np.float32(1e-8)),
                           np.float32(0.0), qmax)).astype(np.float32)
    return x_q * scale + x_min


def rope_tables():
    inv_freq = (1.0 / 10000.0 ** (np.arange(0, D, 2, dtype=np.float32) / D)).astype(np.float32)
    t = np.arange(S, dtype=np.float32)
    freqs = t[:, None].astype(np.float32) * inv_freq[None, :]
    sin = np.sin(freqs).astype(np.float32)   # (S, 32)
    cos = np.cos(freqs).astype(np.float32)
    cosT = np.tile(np.ascontiguousarray(cos.T), (4, 1))  # (128, S), [d, s]
    sinT = np.tile(np.ascontiguousarray(sin.T), (4, 1))
    return cosT, sinT


def build_kernel(debug=False):
    nc = bacc.Bacc(trn_type="TRN2")
    dbg = {}
    if debug:
        for name, shape, dt in [
                ("d_qc0", [128, S], F32), ("d_kc0", [128, S], F32),
                ("d_va0", [128, HL * (D + 1)], F32), ("d_p000", [128, 1024], F32),
                ("d_uraw00", [D + 1, 512], F32), ("d_un20", [128, S], F32),
                ("d_xt0", [128, SH], F32)]:
            dbg[name] = nc.declare_dram_parameter(name, shape, dt, isOutput=True)
    xt = nc.declare_dram_parameter("xt", [E, S], BF16, isOutput=False)
    wqk = nc.declare_dram_parameter("wqk", [4, 128, ET * 128], BF16, isOutput=False)
    wv = nc.declare_dram_parameter("wv", [128, ET * HL * D], BF16, isOutput=False)
    wout = nc.declare_dram_parameter("wout", [2, 128, E], F32R, isOutput=False)
    cost = nc.declare_dram_parameter("cost", [128, S], F32, isOutput=False)
    sint = nc.declare_dram_parameter("sint", [128, S], F32, isOutput=False)
    ypart = nc.declare_dram_parameter("ypart", [S, E], F32, isOutput=True)

    with tc_pools(nc) as (tc, sb, ps):
        # ---------------- constants / weights / x loads in consumption order
        ones68 = sb.tile([D + 4, D], F32, tag="ones", bufs=1)
        nc.vector.memset(ones68, 1.0)

        wqk_sb = {}
        for ct in (0, 1):
            t = sb.tile([128, ET * 128], BF16, tag="wqk", bufs=4, name=f"wqk{ct}")
            nc.sync.dma_start(out=t, in_=wqk[ct, :, :])
            wqk_sb[ct] = t
        xT = []
        for g in range(ET):       # first s-half
            t = sb.tile([128, SH], BF16, tag="xT", bufs=2 * ET, name=f"xT{g}")
            nc.sync.dma_start(out=t, in_=xt[g * 128:(g + 1) * 128, 0:SH])
            xT.append(t)
        cos_sb = sb.tile([128, S], F32, tag="trig", bufs=2)
        sin_sb = sb.tile([128, S], F32, tag="trig", bufs=2)
        nc.scalar.dma_start(out=cos_sb, in_=cost[:, :])
        nc.scalar.dma_start(out=sin_sb, in_=sint[:, :])
        for ct in (2, 3):
            t = sb.tile([128, ET * 128], BF16, tag="wqk", bufs=4, name=f"wqk{ct}")
            nc.scalar.dma_start(out=t, in_=wqk[ct, :, :])
            wqk_sb[ct] = t
        wv_sb = sb.tile([128, ET * HL * D], BF16, tag="wv", bufs=1)
        nc.scalar.dma_start(out=wv_sb, in_=wv[:, :])
        for g in range(ET, 2 * ET):   # second s-half
            t = sb.tile([128, SH], BF16, tag="xT", bufs=2 * ET, name=f"xT{g}")
            nc.sync.dma_start(out=t, in_=xt[g % ET * 128:(g % ET + 1) * 128, SH:S])
            xT.append(t)

        # head-contiguous rotated q/k, stacked normalized outputs, v tiles
        qcont = [sb.tile([128, S], BF16, tag="qk", bufs=4, name=f"qcont{p}")
                 for p in range(2)]
        kcont = [sb.tile([128, S], BF16, tag="qk", bufs=4, name=f"kcont{p}")
                 for p in range(2)]
        u_norm2 = [sb.tile([128, S], F32R, tag="un2", bufs=2, name=f"un2_{hp}")
                   for hp in range(2)]
        v_aug = []

        # ---------------- phases 1-2 per s-half: q/k proj + RoPE, v proj
        for sh in range(S // SH):
            ssl = slice(sh * SH, (sh + 1) * SH)
            xh = xT[sh * ET:(sh + 1) * ET]

            for pair, dest in ((0, "q"), (2, "k")):
                rot = [sb.tile([128, SH], BF16, tag="rot", bufs=4,
                               name=f"rot{dest}{half}{sh}") for half in range(2)]
                for q2 in range(SH // 512):
                    qsl = slice(q2 * 512, (q2 + 1) * 512)
                    gsl = slice(sh * SH + q2 * 512, sh * SH + (q2 + 1) * 512)
                    bb = ps.tile([128, 1024], F32, tag="sc", bufs=2)
                    b1, b2 = bb[:, 0:512], bb[:, 512:1024]
                    for et in range(ET):
                        nc.tensor.matmul(b1, wqk_sb[pair][:, et * 128:(et + 1) * 128],
                                         xh[et][:, qsl],
                                         start=(et == 0), stop=(et == ET - 1))
                    for et in range(ET):
                        nc.tensor.matmul(b2, wqk_sb[pair + 1][:, et * 128:(et + 1) * 128],
                                         xh[et][:, qsl],
                                         start=(et == 0), stop=(et == ET - 1))
                    t1 = sb.tile([128, 512], F32, tag="t1", bufs=2)
                    t2 = sb.tile([128, 512], F32, tag="t2", bufs=2)
                    nc.vector.tensor_mul(t1, b1, cos_sb[:, gsl])
                    nc.vector.tensor_mul(t2, b2, sin_sb[:, gsl])
                    nc.vector.tensor_sub(rot[0][:, qsl], t1, t2)
                    t3 = sb.tile([128, 512], F32, tag="t1", bufs=2)
                    t4 = sb.tile([128, 512], F32, tag="t2", bufs=2)
                    nc.vector.tensor_mul(t3, b1, sin_sb[:, gsl])
                    nc.vector.tensor_mul(t4, b2, cos_sb[:, gsl])
                    nc.vector.tensor_add(rot[1][:, qsl], t3, t4)
                cont = qcont if dest == "q" else kcont
                for h in range(HL):
                    p, j = divmod(h, 2)
                    for half in range(2):
                        rows_out = slice(64 * j + 32 * half, 64 * j + 32 * half + 32)
                        nc.gpsimd.dma_start(
                            out=cont[p][rows_out, ssl],
                            in_=rot[half][32 * h:32 * h + 32, :])

            # v projection (natural [s, d]) + ones column
            for st_l in range(SH // 128):
                st = sh * (SH // 128) + st_l
                pv = ps.tile([128, HL * D], F32, tag="pv", bufs=4)
                for et in range(ET):
                    nc.tensor.matmul(pv, xh[et][:, st_l * 128:(st_l + 1) * 128],
                                     wv_sb[:, et * HL * D:(et + 1) * HL * D],
                                     start=(et == 0), stop=(et == ET - 1))
                va = sb.tile([128, HL, D + 1], BF16, tag="vaug", bufs=ST,
                             name=f"vaug{st}")
                nc.vector.memset(va[:, :, D:D + 1], 1.0)
                nc.vector.tensor_copy(va[:, :, 0:D],
                                      pv.rearrange("p (h d) -> p h d", h=HL))
                if debug and st == 0:
                    vaf = sb.tile([128, HL * (D + 1)], F32, tag="dbgsmall", bufs=1)
                    nc.vector.tensor_copy(vaf, va.rearrange("p a b -> p (a b)"))
                    nc.sync.dma_start(out=dbg["d_va0"][:, :], in_=vaf)
                v_aug.append(va)

        # output-projection weights: issue now, transfer during attention
        wo2 = {}
        for hp in range(2):
            for ec in range(2):
                t = sb.tile([128, 512], F32R, tag="wo", bufs=4, name=f"wo{hp}_{ec}")
                nc.scalar.dma_start(out=t, in_=wout[hp, :, ec * 512:(ec + 1) * 512])
                wo2[(hp, ec)] = t

        # ---------------- phase 3: attention
        # qchunk 512, PVs skewed one kt behind the scores so exps hide
        # under the next kt's scores and the in-order PE queue never stalls.
        # U accumulators evicted to SBUF right after their last PV (8 PSUM
        # banks: 2x scores [128,1024] + 4x U [65,512]); normalization is
        # spread over kts 6..9 of the following q-chunk.
        from concourse.tile_rust import add_dep_helper

        def do_norm(h, qc, u_raw):
            qsl = slice(qc * 512, (qc + 1) * 512)
            bc = ps.tile([D, 512], F32, tag="sc", bufs=2)
            nc.tensor.matmul(bc, ones68[D:D + 1, :],
                             u_raw[D:D + 1, :].bitcast(F32),
                             start=True, stop=True)
            hp, half = divmod(h, 2)
            if half == 0:
                nc.vector.tensor_mul(u_norm2[hp][0:D, qsl], u_raw[0:D, :], bc)
            else:
                u_tmp = sb.tile([D, 512], F32R, tag="utmp", bufs=2)
                nc.vector.tensor_mul(u_tmp, u_raw[0:D, :], bc)
                nc.gpsimd.dma_start(out=u_norm2[hp][D:2 * D, qsl], in_=u_tmp)

        skewed = None      # (qc, upsum, p_ts of previous kt)
        to_evict = None    # (qc, upsum) finished accumulating
        to_finish = []     # (h, qc, u_raw) awaiting normalization
        last_pv = None
        for qc in range(NQ):
            qsl = slice(qc * 512, (qc + 1) * 512)
            upsum = {}
            for kt in range(KT):
                # scores + exp for (qc, kt)
                p_ts = {}
                s_last = None
                for pr in range(2):
                    s_ps = ps.tile([128, 1024], F32, tag="sc", bufs=2)
                    for j in range(2):
                        mm = nc.tensor.matmul(
                            s_ps[:, j * 512:(j + 1) * 512],
                            kcont[pr][64 * j:64 * j + 64, kt * 128:(kt + 1) * 128],
                            qcont[pr][64 * j:64 * j + 64, qsl],
                            start=True, stop=True)
                        if last_pv is not None:
                            add_dep_helper(mm.ins, last_pv.ins, sync=False,
                                           reason="pe order")
                        s_last = mm
                    p_t = sb.tile([128, 1024], BF16, tag="p", bufs=4)
                    nc.scalar.activation(p_t, s_ps, AF.Exp, scale=0.125)
                    if debug and qc == 0 and kt == 0 and pr == 0:
                        ptf = sb.tile([128, 1024], F32, tag="dbgbig", bufs=1)
                        nc.vector.tensor_copy(ptf, p_t)
                        nc.sync.dma_start(out=dbg["d_p000"][:, :], in_=ptf)
                    p_ts[pr] = p_t
                # evict the previous q-chunk's finished U accumulators,
                # reciprocal of the denominator row in place (fast approx)
                if to_evict is not None:
                    eqc, eup = to_evict
                    gath = sb.tile([D + 4, 512], F32, tag="gath", bufs=2,
                                   name=f"gath{eqc}")
                    for h in range(HL):
                        u_raw = sb.tile([D + 1, 512], F32R, tag="uraw", bufs=8,
                                        name=f"uraw{h}_{eqc}")
                        nc.vector.tensor_copy(u_raw, eup[h])
                        nc.gpsimd.dma_start(out=gath[D + h:D + h + 1, :],
                                            in_=u_raw[D:D + 1, :].bitcast(F32))
                        to_finish.append((h, eqc, u_raw))
                    nc.vector.reciprocal(gath[D:D + 4, :], gath[D:D + 4, :])
                    for h in range(HL):
                        nc.gpsimd.dma_start(
                            out=to_finish[-HL + h][2][D:D + 1, :].bitcast(F32),
                            in_=gath[D + h:D + h + 1, :])
                    if debug and eqc == 0:
                        urf = sb.tile([D + 1, 512], F32, tag="dbgsmall", bufs=1)
                        nc.vector.tensor_copy(urf[D:D + 1, :], gath[D:D + 1, :])
                        nc.sync.dma_start(out=dbg["d_uraw00"][:, :], in_=urf)
                    to_evict = None
                # skewed PV batch for (qc, kt-1) / (qc-1, KT-1)
                if skewed is not None:
                    sqc, sup, sp = skewed
                    if not sup:
                        for h in range(HL):
                            sup[h] = ps.tile([D + 1, 512], F32, tag="pv",
                                             bufs=4, name=f"u{h}_{sqc}")
                    for h in range(HL):
                        mm = nc.tensor.matmul(
                            sup[h], v_aug[sp[1]][:, h, :],
                            sp[0][h // 2][:, (h % 2) * 512:(h % 2) * 512 + 512],
                            start=(sp[1] == 0), stop=(sp[1] == KT - 1))
                        add_dep_helper(mm.ins, s_last.ins, sync=False,
                                       reason="pe order")
                        last_pv = mm
                    if sp[1] == KT - 1:
                        to_evict = (sqc, sup)
                # deferred normalizations, one per kt, far from their recips
                if kt >= 6 and to_finish:
                    do_norm(*to_finish.pop(0))
                skewed = (qc, upsum, (p_ts, kt))
        # drain: last PV batch, evict, normalize
        sqc, sup, sp = skewed
        for h in range(HL):
            nc.tensor.matmul(
                sup[h], v_aug[sp[1]][:, h, :],
                sp[0][h // 2][:, (h % 2) * 512:(h % 2) * 512 + 512],
                start=False, stop=True)
        gath = sb.tile([D + 4, 512], F32, tag="gath", bufs=2, name=f"gath{sqc}")
        for h in range(HL):
            u_raw = sb.tile([D + 1, 512], F32R, tag="uraw", bufs=8,
                            name=f"uraw{h}_{sqc}")
            nc.vector.tensor_copy(u_raw, sup[h])
            nc.gpsimd.dma_start(out=gath[D + h:D + h + 1, :],
                                in_=u_raw[D:D + 1, :].bitcast(F32))
            to_finish.append((h, sqc, u_raw))
        nc.vector.reciprocal(gath[D:D + 4, :], gath[D:D + 4, :])
        for h in range(HL):
            nc.gpsimd.dma_start(
                out=to_finish[-HL + h][2][D:D + 1, :].bitcast(F32),
                in_=gath[D + h:D + h + 1, :])
        for args in to_finish:
            do_norm(*args)

        if debug:
            qcf = sb.tile([128, S], F32, tag="dbgbig", bufs=1)
            nc.vector.tensor_copy(qcf, qcont[0])
            nc.sync.dma_start(out=dbg["d_qc0"][:, :], in_=qcf)
            kcf = sb.tile([128, S], F32, tag="dbgbig", bufs=1)
            nc.vector.tensor_copy(kcf, kcont[0])
            nc.sync.dma_start(out=dbg["d_kc0"][:, :], in_=kcf)
            un2f = sb.tile([128, S], F32, tag="dbgbig", bufs=1)
            nc.vector.tensor_copy(un2f, u_norm2[0].bitcast(F32))
            nc.sync.dma_start(out=dbg["d_un20"][:, :], in_=un2f)
            xtf = sb.tile([128, SH], F32, tag="dbgbig", bufs=1)
            nc.vector.tensor_copy(xtf, xT[0])
            nc.sync.dma_start(out=dbg["d_xt0"][:, :], in_=xtf)

        # ---------------- phase 4: output projection (partial), K=128 pairs
        for st in range(ST):
            stsl = slice(st * 128, (st + 1) * 128)
            y_ps = ps.tile([128, 1024], F32, tag="sc", bufs=2)
            for ec in range(2):
                for hp in range(2):
                    nc.tensor.matmul(y_ps[:, ec * 512:(ec + 1) * 512],
                                     u_norm2[hp][:, stsl], wo2[(hp, ec)],
                                     start=(hp == 0), stop=(hp == 1))
            y_sb = sb.tile([128, 1024], F32, tag="ysb", bufs=4)
            if st % 2 == 0:
                nc.vector.tensor_copy(y_sb, y_ps)
            else:
                nc.scalar.copy(y_sb, y_ps)
            nc.gpsimd.dma_start(out=ypart[stsl, :], in_=y_sb)
    nc.finalize()
    return nc


class tc_pools:
    """TileContext plus the two pools, as one context manager."""
    def __init__(self, nc):
        self.nc = nc

    def __enter__(self):
        import contextlib
        self._stack = contextlib.ExitStack()
        tc = self._stack.enter_context(tile.TileContext(self.nc))
        sb = self._stack.enter_context(tc.tile_pool(name="sb", bufs=1))
        ps = self._stack.enter_context(tc.tile_pool(name="ps", bufs=2, space="PSUM"))
        return tc, sb, ps

    def __exit__(self, *exc):
        return self._stack.__exit__(*exc)


def make_inputs(x, w_qkv, w_out):
    """Host-side prep: quantize, cast, split/re-layout per core."""
    x = np.asarray(x, dtype=np.float32)
    wq_deq = quantize_bits_np(np.asarray(w_qkv, dtype=np.float32))
    wo_deq = round_f32r(quantize_bits_np(np.asarray(w_out, dtype=np.float32)))
    cosT, sinT = rope_tables()

    x_t = [np.ascontiguousarray(x[b].T).astype(ml_dtypes.bfloat16) for b in range(B)]

    in_maps = []
    for c in range(8):
        b, hg = divmod(c, 4)
        heads = [hg * HL + i for i in range(HL)]
        # interleaved q/k col-tiles, packed partition-major:
        # wqk_host[ct][p, et*128 + cc] = wq_deq[et*128 + p, col[cc]]
        wqk_t = np.empty((4, 128, ET * 128), dtype=np.float32)
        for half in range(2):
            cols = np.concatenate(
                [np.arange(h * D + 32 * half, h * D + 32 * half + 32) for h in heads])
            wqk_t[0 + half] = (wq_deq[:, 0 * E + cols]
                               .reshape(ET, 128, 128).transpose(1, 0, 2)
                               .reshape(128, ET * 128))
            wqk_t[2 + half] = (wq_deq[:, 1 * E + cols]
                               .reshape(ET, 128, 128).transpose(1, 0, 2)
                               .reshape(128, ET * 128))
        vcols = np.concatenate([np.arange(h * D, h * D + D) for h in heads])
        wv_t = (wq_deq[:, 2 * E + vcols]
                .reshape(ET, 128, HL * D).transpose(1, 0, 2)
                .reshape(128, ET * HL * D))
        wout_t = np.stack([
            np.concatenate([wo_deq[heads[2 * hp] * D:(heads[2 * hp] + 1) * D, :],
                            wo_deq[heads[2 * hp + 1] * D:(heads[2 * hp + 1] + 1) * D, :]],
                           axis=0)
            for hp in range(2)])
        in_maps.append({
            "xt": x_t[b],
            "wqk": wqk_t.astype(ml_dtypes.bfloat16),
            "wv": wv_t.astype(ml_dtypes.bfloat16),
            "wout": wout_t,
            "cost": cosT, "sint": sinT,
        })
    return in_maps


_NC_CACHE = {}


def get_nc():
    if "nc" not in _NC_CACHE:
        _NC_CACHE["nc"] = build_kernel()
    return _NC_CACHE["nc"]


def kernel(x, w_qkv, w_out):
    from concourse.bass_utils import run_bass_kernel_spmd
    nc = get_nc()
    in_maps = make_inputs(x, w_qkv, w_out)
    res = run_bass_kernel_spmd(nc, in_maps, list(range(8)))
    out = np.zeros((B, S, E), dtype=np.float32)
    for c in range(8):
        out[c // 4] += res.results[c]["ypart"]
    return out
